# revision 17
# baseline (speedup 1.0000x reference)
"""GATv2Conv on 8 Trainium2 NeuronCores (Bass/Tile).

Strategy (edge-parallel by destination):
- Host: project x@W (fp32 BLAS), interleave channels head-minor (c=4c'+h),
  cast bf16. Sort edges by destination; shard dst nodes contiguously over 8
  cores. Per core, pack edges into uniform "windows": each window is 16 tiles
  of 128 edge slots = 4 chunk-segments of 512 slots (srcs bucketed by table
  chunk of 32768 rows so int16 gather indices work). Windows cover <=128
  consecutive dst nodes. All structure is identical across cores (SPMD): the
  per-core node shard is handled by uploading a rotated copy of the node
  table, so dst indices are core-local.
- Device per window: SWDGE dma_gather src rows (4 chunk calls per 5-window
  batch) + dst rows (1 call); u = src+dst (DVE); transpose u tiles on PE;
  tanh (ACT, PSUM->SBUF); score s[e,h] via PE (stationary t^T x att mask);
  w = exp(s) (ACT); one-hot Q from col_local (DVE tensor_scalar is_equal);
  msg = src*w (DVE, head-minor broadcast); segment-sum via PE matmul
  Q^T @ [msg|w] accumulated in PSUM per window; normalize; DMA out.
- No collectives: output is dst-sharded; host reassembles.

Math note: the reference's global-max shift cancels in out/normalizer, so we
use unshifted exp(s); fp32 range is ample (|s| <= ~26).
"""

import numpy as np

# ---------------------------------------------------------------- constants
N_NODES = 100000
N_EDGES = 1600000
IN_CH = 128
HEADS = 4
OUT_CH = 32
HC = HEADS * OUT_CH          # 128

NCORES = 8
NPC = N_NODES // NCORES      # 12500 dst nodes per core
SLICE_ROWS = 12800           # per-core table slice rows (padded, AllGather unit)
GTAB_ROWS = NCORES * SLICE_ROWS   # 102400
CHUNK = 32768                # src gather chunk (int16 index limit)
NCHUNKS = 4
SEG = 640                    # slots per (window, chunk) segment
WTILES = 20                  # tiles per window (= NCHUNKS*SEG/128)
WSLOTS = SEG * NCHUNKS       # 2048 edge slots per window
BATCH_W = 5                  # windows gathered per SWDGE batch
P = 128

_cache = {}


# ---------------------------------------------------------------- host prep
def _build_windows(col_rel, row, core_base):
    """Greedy windows over this core's dst-sorted edges (vectorized).

    Cut a window when any src-chunk segment would exceed SEG or the window
    would span more than 128 dst nodes. Returns (windows, chunk, srcloc):
    windows = list of (nstart, edge_lo, edge_hi).
    """
    nE = col_rel.shape[0]
    gl = (row // NPC) * SLICE_ROWS + (row % NPC)
    srcloc = gl & (CHUNK - 1)
    chunk = gl >> 15

    # per-(node, chunk) counts -> exclusive prefix over nodes
    cnt = np.bincount(col_rel * NCHUNKS + chunk,
                      minlength=NPC * NCHUNKS).reshape(NPC, NCHUNKS)
    Cx = np.zeros((NPC + 1, NCHUNKS), dtype=np.int64)
    np.cumsum(cnt, axis=0, out=Cx[1:])
    # edge index of first edge of each node
    estart = np.zeros(NPC + 1, dtype=np.int64)
    np.cumsum(cnt.sum(axis=1), out=estart[1:])

    windows = []
    s = 0
    while s < NPC:
        e = min(s + P, NPC)
        for c in range(NCHUNKS):
            ec = np.searchsorted(Cx[:, c], Cx[s, c] + SEG, side="right") - 1
            e = min(e, int(ec))
        assert e > s
        if estart[e] > estart[s]:
            windows.append((s, int(estart[s]), int(estart[e])))
        s = e
    return windows, chunk, srcloc


def _prep_core(core, row_s, col_s, lo, hi, W_COUNT, prebuilt=None):
    core_base = core * NPC
    row = row_s[lo:hi]
    col_rel = col_s[lo:hi] - core_base
    if prebuilt is None:
        windows, chunk, srcloc = _build_windows(col_rel, row, core_base)
    else:
        windows, chunk, srcloc = prebuilt
    nW = len(windows)
    assert nW <= W_COUNT, (core, nW, W_COUNT)

    S = W_COUNT * WSLOTS
    srcidx = np.zeros(S, dtype=np.int16)
    dstidx = np.zeros(S, dtype=np.int16)
    colloc = np.full(S, -1.0, dtype=np.float32)
    nstarts = np.zeros(W_COUNT, dtype=np.int64)

    nE = col_rel.shape[0]
    # per-edge window id + window node start (vectorized over windows list)
    wlo = np.array([w[1] for w in windows], dtype=np.int64)
    wns = np.array([w[0] for w in windows], dtype=np.int64)
    nstarts[:nW] = wns
    wid = np.searchsorted(wlo, np.arange(nE), side="right") - 1

    # group = (window, chunk); edges are processed in order, so the rank of
    # an edge within its group = arange - first_index_of_group, computed via
    # a stable ordering by group id.
    gid = wid * NCHUNKS + chunk
    order = np.argsort(gid, kind="stable")
    gs = gid[order]
    first_of_group = np.concatenate([[0], np.flatnonzero(gs[1:] != gs[:-1]) + 1])
    glen = np.diff(np.concatenate([first_of_group, [nE]]))
    pos_sorted = np.arange(nE) - np.repeat(first_of_group, glen)
    pos = np.empty(nE, dtype=np.int64)
    pos[order] = pos_sorted

    b = wid // BATCH_W
    wb = wid - b * BATCH_W
    nwb = np.minimum(BATCH_W, W_COUNT - b * BATCH_W)
    slot = (b * BATCH_W * WSLOTS) + chunk * (nwb * SEG) + wb * SEG + pos

    srcidx[slot] = srcloc.astype(np.int16)
    dstidx[slot] = col_rel.astype(np.int16)
    colloc[slot] = (col_rel - wns[wid]).astype(np.float32)

    # wrap for device:
    # idx arrays: slot j -> [j%16, j//16], replicated to 128 partitions
    def wrap16(a):
        return np.ascontiguousarray(a.reshape(-1, 16).T)  # [16, S/16]
    # col: slot j=(t*128+p) -> [p, t]
    import ml_dtypes
    colw = np.ascontiguousarray(colloc.reshape(-1, P).T).astype(ml_dtypes.bfloat16)

    return wrap16(srcidx), wrap16(dstidx), colw, nstarts, nW


def _host_prep(x, edge_index, W, att):
    import ml_dtypes

    proj = (np.asarray(x, dtype=np.float32) @ np.asarray(W, dtype=np.float32))
    # head-minor channel interleave: new col 4*c' + h  <- old col h*32 + c'
    perm = (np.arange(HC) % OUT_CH) * HEADS + (np.arange(HC) // OUT_CH)
    # perm maps old index -> new position; build gather order for columns:
    inv = np.empty(HC, dtype=np.int64)
    inv[perm] = np.arange(HC)
    proj_i = proj[:, inv]                       # new[:, k] = old[:, inv[k]]
    table = proj_i.astype(ml_dtypes.bfloat16)   # [N, 128] bf16

    att = np.asarray(att, dtype=np.float32)
    # attmask_i[c_new, h] = att[h, c'] if c_new % 4 == h else 0
    attmask = np.zeros((HC, HEADS), dtype=np.float32)
    for cn in range(HC):
        h = cn % HEADS
        attmask[cn, h] = att[h, cn // HEADS]

    row = edge_index[0].astype(np.int64)
    col = edge_index[1].astype(np.int64)
    order = np.argsort(col, kind="stable")
    row_s = row[order].astype(np.int64)
    col_s = col[order].astype(np.int64)
    bounds = np.searchsorted(col_s, np.arange(0, N_NODES + 1, NPC))

    # first pass: window counts per core
    per_core = []
    maxW = 0
    for c in range(NCORES):
        lo, hi = bounds[c], bounds[c + 1]
        pb = _build_windows(col_s[lo:hi] - c * NPC, row_s[lo:hi], c * NPC)
        maxW = max(maxW, len(pb[0]))
        per_core.append((lo, hi, pb))
    W_COUNT = maxW

    cores = []
    for c in range(NCORES):
        lo, hi, pb = per_core[c]
        srcw, dstw, colw, nstarts, nW = _prep_core(c, row_s, col_s, lo, hi,
                                                   W_COUNT, prebuilt=pb)
        tsl = np.zeros((SLICE_ROWS, HC), dtype=table.dtype)
        tsl[:NPC] = table[c * NPC:(c + 1) * NPC]
        cores.append(dict(srcidx=srcw, dstidx=dstw, colloc=colw,
                          tslice=tsl, nstarts=nstarts, nW=nW))

    iota = np.broadcast_to(np.arange(P, dtype=np.float32), (P, P))
    consts = dict(
        iota=np.ascontiguousarray(iota).astype(ml_dtypes.bfloat16),
        attmask=attmask.astype(ml_dtypes.bfloat16),
        identity=np.eye(P, dtype=np.float32).astype(ml_dtypes.bfloat16),
    )
    return cores, consts, W_COUNT, perm


# ---------------------------------------------------------------- device
def _build_nc(W_COUNT):
    import concourse.bass as bass
    import concourse.bacc as bacc
    import concourse.mybir as mybir
    import concourse.tile as tile
    from concourse.library_config import mlp

    S = W_COUNT * WSLOTS
    fp32 = mybir.dt.float32
    bf16 = mybir.dt.bfloat16
    i16 = mybir.dt.int16

    nc = bacc.Bacc("TRN2", target_bir_lowering=False, debug=False,
                   num_devices=NCORES)
    tslice = nc.dram_tensor("tslice", [SLICE_ROWS, HC], bf16, kind="ExternalInput")
    srcidx = nc.dram_tensor("srcidx", [16, S // 16], i16, kind="ExternalInput")
    dstidx = nc.dram_tensor("dstidx", [16, S // 16], i16, kind="ExternalInput")
    colloc = nc.dram_tensor("colloc", [P, S // P], bf16, kind="ExternalInput")
    iota_in = nc.dram_tensor("iota", [P, P], bf16, kind="ExternalInput")
    attmask_in = nc.dram_tensor("attmask", [HC, HEADS], bf16, kind="ExternalInput")
    ident_in = nc.dram_tensor("identity", [P, P], bf16, kind="ExternalInput")
    out = nc.dram_tensor("out", [W_COUNT * P, HC], bf16, kind="ExternalOutput")
    tcopy = nc.dram_tensor("tcopy", [SLICE_ROWS, HC], bf16)
    table = nc.dram_tensor("gtable", [GTAB_ROWS, HC], bf16)

    n_batches = (W_COUNT + BATCH_W - 1) // BATCH_W

    with tile.TileContext(nc) as tc:
        with tc.tile_pool(name="consts", bufs=1) as cpool, \
             tc.tile_pool(name="gather", bufs=2) as gpool, \
             tc.tile_pool(name="win", bufs=2) as wpool, \
             tc.tile_pool(name="small", bufs=3) as spool, \
             tc.tile_pool(name="psA", bufs=2, space="PSUM") as psA, \
             tc.tile_pool(name="psS", bufs=2, space="PSUM") as psS, \
             tc.tile_pool(name="psO", bufs=2, space="PSUM") as psO:

            nc.gpsimd.load_library(mlp)

            # AllGather the node table from per-core slices
            nc.sync.dma_start(tcopy[:], tslice[:])
            nc.gpsimd.collective_compute(
                "AllGather", mybir.AluOpType.bypass,
                replica_groups=[list(range(NCORES))],
                ins=[tcopy[:]], outs=[table[:]],
            )

            iota_t = cpool.tile([P, P], bf16, tag="iota")
            attm_t = cpool.tile([HC, HEADS], bf16, tag="attm")
            ident_t = cpool.tile([P, P], bf16, tag="ident")
            nc.sync.dma_start(iota_t[:], iota_in[:])
            nc.sync.dma_start(attm_t[:], attmask_in[:])
            nc.sync.dma_start(ident_t[:], ident_in[:])

            for b in range(n_batches):
                w0 = b * BATCH_W
                nwb = min(BATCH_W, W_COUNT - w0)
                bslots = nwb * WSLOTS
                btiles = bslots // P
                bbase = w0 * WSLOTS

                # gather src (4 chunk calls) + dst (1 call) for this batch
                sbuf = gpool.tile([P, btiles, HC], bf16, tag="S")
                dbuf = gpool.tile([P, btiles, HC], bf16, tag="D")
                colb16 = gpool.tile([P, btiles], bf16, tag="col16")
                colb = gpool.tile([P, btiles], fp32, tag="col")
                sidx_t = gpool.tile([P, bslots // 16], i16, tag="sidx")
                didx_t = gpool.tile([P, bslots // 16], i16, tag="didx")
                nc.sync.dma_start(colb16[:], colloc[:, bbase // P: bbase // P + btiles])
                nc.vector.tensor_copy(colb[:], colb16[:])
                nc.sync.dma_start(
                    sidx_t[:],
                    srcidx[:, bbase // 16: (bbase + bslots) // 16]
                    .unsqueeze(0).broadcast_to([8, 16, bslots // 16]))
                nc.sync.dma_start(
                    didx_t[:],
                    dstidx[:, bbase // 16: (bbase + bslots) // 16]
                    .unsqueeze(0).broadcast_to([8, 16, bslots // 16]))

                segslots = nwb * SEG
                for c in range(NCHUNKS):
                    i0 = c * segslots
                    rows = CHUNK if c < NCHUNKS - 1 else GTAB_ROWS - 3 * CHUNK
                    nc.gpsimd.dma_gather(
                        sbuf[:, c * (segslots // P): (c + 1) * (segslots // P), :],
                        table[c * CHUNK: c * CHUNK + rows, :],
                        sidx_t[:, i0 // 16: (i0 + segslots) // 16],
                        segslots, segslots, HC,
                        single_packet=False,
                    )
                nc.gpsimd.dma_gather(
                    dbuf[:],
                    tslice[0:NPC, :],
                    didx_t[:],
                    bslots, bslots, HC,
                    single_packet=False,
                )
                # u = src + dst (overwrite dbuf)
                nc.vector.tensor_add(dbuf[:], dbuf[:], sbuf[:])

                for wb in range(nwb):
                    w = w0 + wb
                    # tile indices of this window inside the batch buffers
                    wtiles = [c * (segslots // P) + wb * (SEG // P) + t
                              for c in range(NCHUNKS) for t in range(SEG // P)]

                    tT = wpool.tile([P, WTILES, P], bf16, tag="tT")
                    s_ps = psS.tile([P, WTILES, HEADS], fp32, tag="s")
                    # transpose u tiles in groups of 4 -> tanh -> tT
                    for g in range(WTILES // 4):
                        uT = psA.tile([P, 4, P], bf16, tag="uT")
                        for t4 in range(4):
                            gt = wtiles[g * 4 + t4]
                            nc.tensor.transpose(
                                uT[:, t4, :], dbuf[:, gt, :], ident_t[:])
                        nc.scalar.activation(
                            tT[:, g * 4:(g + 1) * 4, :], uT[:],
                            mybir.ActivationFunctionType.Tanh)
                    # score per tile: s[e,h] = sum_c tT[c,e]*attmask[c,h]
                    for t in range(WTILES):
                        nc.tensor.matmul(
                            s_ps[:, t, :], tT[:, t, :], attm_t[:],
                            start=True, stop=True)

                    msgw = wpool.tile([P, WTILES, 136], bf16, tag="msgw")
                    # w = exp(s) -> msgw[:, :, 128:132]
                    nc.scalar.activation(
                        msgw[:, :, HC:HC + HEADS], s_ps[:],
                        mybir.ActivationFunctionType.Exp)
                    # msg = src * w (head-minor broadcast of w)
                    wview = msgw[:, :, HC:HC + HEADS]  # [P, WTILES, 4]
                    for c in range(NCHUNKS):
                        g0 = c * (SEG // P)
                        src_g = sbuf[:, wtiles[g0]: wtiles[g0] + SEG // P, :]
                        nc.vector.tensor_mul(
                            msgw[:, g0:g0 + SEG // P, 0:HC],
                            src_g,
                            wview[:, g0:g0 + SEG // P, :]
                            .unsqueeze(2)
                            .broadcast_to([P, SEG // P, OUT_CH, HEADS]),
                        )

                    # scatter: out_acc[n, :] += sum_e Q[e,n]*msgw[e, :]
                    out_ps = psO.tile([P, 132], fp32, tag="acc")
                    for t in range(WTILES):
                        q = spool.tile([P, P], bf16, tag="q")
                        nc.vector.tensor_scalar(
                            q[:], iota_t[:], colb[:, wtiles[t]:wtiles[t] + 1],
                            None, op0=mybir.AluOpType.is_equal)
                        nc.tensor.matmul(
                            out_ps[:], q[:], msgw[:, t, 0:132],
                            start=(t == 0), stop=(t == WTILES - 1))

                    # normalize: out = num / max(norm, 1e-30)
                    rec = spool.tile([P, HEADS], fp32, tag="rec")
                    nc.vector.tensor_scalar(
                        rec[:], out_ps[:, HC:HC + HEADS], 1e-30, None,
                        op0=mybir.AluOpType.max)
                    nc.vector.reciprocal(rec[:], rec[:])
                    o_sb = spool.tile([P, HC], bf16, tag="osb")
                    nc.vector.tensor_mul(
                        o_sb[:], out_ps[:, 0:HC],
                        rec[:].unsqueeze(1)
                        .broadcast_to([P, OUT_CH, HEADS]))
                    nc.sync.dma_start(out[w * P:(w + 1) * P, :], o_sb[:])

    nc.compile()
    return nc


# ---------------------------------------------------------------- entry
LAST_EXEC_NS = None


def _get_runner(nc):
    """Build (once) a cached jitted SPMD executor for this Bass module,
    mirroring bass2jax.run_bass_via_pjrt but with: (a) the jit closure
    cached across calls, (b) output buffers created device-side instead of
    uploading host zeros."""
    import jax
    import jax.numpy as jnp
    from jax.experimental.shard_map import shard_map
    from jax.sharding import Mesh, PartitionSpec
    import concourse.mybir as mybir
    from concourse import bass2jax

    bass2jax.install_neuronx_cc_hook()

    in_names, out_names, out_avals = [], [], []
    for alloc in nc.m.functions[0].allocations:
        if not isinstance(alloc, mybir.MemoryLocationSet):
            continue
        name = alloc.memorylocations[0].name
        if alloc.kind == "ExternalInput":
            if nc.partition_id_tensor is None or name != nc.partition_id_tensor.name:
                in_names.append(name)
        elif alloc.kind == "ExternalOutput":
            out_names.append(name)
            out_avals.append(jax.core.ShapedArray(
                tuple(alloc.tensor_shape), mybir.dt.np(alloc.dtype)))
    n_params = len(in_names)
    all_names = in_names + out_names
    if nc.partition_id_tensor is not None:
        all_names.append(nc.partition_id_tensor.name)

    def _body(*args):
        operands = list(args)
        if nc.partition_id_tensor is not None:
            operands.append(bass2jax.partition_id_tensor())
        outs = bass2jax._bass_exec_p.bind(
            *operands,
            out_avals=tuple(out_avals),
            in_names=tuple(all_names),
            out_names=tuple(out_names),
            lowering_input_output_aliases=(),
            sim_require_finite=True,
            sim_require_nnan=True,
            nc=nc,
        )
        return tuple(outs)

    n_outs = len(out_names)
    devices = jax.devices()[:NCORES]
    mesh = Mesh(np.asarray(devices), ("core",))
    donate = tuple(range(n_params, n_params + n_outs))
    sharded = jax.jit(shard_map(
        _body, mesh=mesh,
        in_specs=(PartitionSpec("core"),) * (n_params + n_outs),
        out_specs=(PartitionSpec("core"),) * n_outs,
        check_rep=False), donate_argnums=donate, keep_unused=True)
    from jax.sharding import NamedSharding
    shard = NamedSharding(mesh, PartitionSpec("core"))

    def put(name, arrays):
        """Async device_put of one input (list of per-core arrays), sharded."""
        import jax
        return jax.device_put(np.concatenate(arrays, axis=0), shard)

    def run(in_maps):
        concat_in = [
            in_maps[0][n + "__dev"] if (n + "__dev") in in_maps[0] else
            np.concatenate([np.asarray(in_maps[c][n]) for c in range(NCORES)], axis=0)
            for n in in_names
        ]
        dev_zeros = [
            jnp.zeros((NCORES * a.shape[0], *a.shape[1:]), a.dtype, device=shard)
            for a in out_avals
        ]
        out_arrs = sharded(*concat_in, *dev_zeros)
        return [
            {n: np.asarray(out_arrs[i]).reshape(NCORES, *out_avals[i].shape)[c]
             for i, n in enumerate(out_names)}
            for c in range(NCORES)
        ]

    run.put = put
    return run


def kernel(x, edge_index, W, att):
    import os
    import ml_dtypes
    import jax
    from jax.sharding import Mesh, PartitionSpec, NamedSharding

    # --- table first, so its upload overlaps the edge preprocessing
    proj = (np.asarray(x, dtype=np.float32) @ np.asarray(W, dtype=np.float32))
    perm = (np.arange(HC) % OUT_CH) * HEADS + (np.arange(HC) // OUT_CH)
    inv = np.empty(HC, dtype=np.int64)
    inv[perm] = np.arange(HC)
    table = np.ascontiguousarray(proj[:, inv]).astype(ml_dtypes.bfloat16)
    ts_all = np.zeros((NCORES * SLICE_ROWS, HC), dtype=table.dtype)
    for c in range(NCORES):
        ts_all[c * SLICE_ROWS: c * SLICE_ROWS + NPC] = table[c * NPC:(c + 1) * NPC]
    ts_dev = None
    if not os.environ.get("GAT_TRACE"):
        devices = jax.devices()[:NCORES]
        mesh = Mesh(np.asarray(devices), ("core",))
        ts_dev = jax.device_put(ts_all, NamedSharding(mesh, PartitionSpec("core")))

    cores, consts, W_COUNT, _perm = _host_prep(x, edge_index, W, att)

    key = ("nc", W_COUNT)
    if key not in _cache:
        _cache[key] = _build_nc(W_COUNT)
    nc = _cache[key]

    in_maps = []
    for c in range(NCORES):
        d = cores[c]
        in_maps.append({
            "tslice": d["tslice"],
            "srcidx": d["srcidx"],
            "dstidx": d["dstidx"],
            "colloc": d["colloc"],
            "iota": consts["iota"],
            "attmask": consts["attmask"],
            "identity": consts["identity"],
        })

    global LAST_EXEC_NS
    if os.environ.get("GAT_TRACE"):
        from concourse.bass_utils import run_bass_kernel_spmd
        res = run_bass_kernel_spmd(nc, in_maps, core_ids=list(range(NCORES)),
                                   trace=True)
        LAST_EXEC_NS = res.exec_time_ns
        results = res.results
    else:
        rkey = ("run", W_COUNT)
        if rkey not in _cache:
            _cache[rkey] = _get_runner(nc)
        if ts_dev is not None:
            in_maps[0]["tslice__dev"] = ts_dev
        results = _cache[rkey](in_maps)

    out = np.zeros((N_NODES, HC), dtype=np.float32)
    for c in range(NCORES):
        d = cores[c]
        dev = results[c]["out"]
        nW = d["nW"]
        ns = d["nstarts"][:nW]
        ne = np.concatenate([ns[1:], [NPC]])
        nn = np.minimum(ne - ns, P)
        tot = int(nn.sum())
        offs = np.arange(tot) - np.repeat(np.concatenate([[0], np.cumsum(nn)[:-1]]), nn)
        dev_rows = np.repeat(np.arange(nW) * P, nn) + offs
        dst_rows = c * NPC + np.repeat(ns, nn) + offs
        out[dst_rows] = dev[dev_rows].astype(np.float32)
    # un-interleave channels: result col (h*32+c') = device col (4c'+h)
    out = out[:, perm]
    return np.ascontiguousarray(out, dtype=np.float32)
